# revision 19
# baseline (speedup 1.0000x reference)
"""Trainium2 Bass kernel for FFNWithScales (SwiGLU MLP with low-rank dequant scales).

Reference computation (all fp32):
    gate_eff = gate_snapped * (gate_scale_A @ gate_scale_B)       # [8192, 2048]
    up_eff   = up_snapped   * (up_scale_A   @ up_scale_B)         # [8192, 2048]
    down_eff = down_snapped * (down_scale_A @ down_scale_B)       # [2048, 8192]
    h   = silu(gate_eff @ x) * (up_eff @ x)                       # [8192, 512]
    out = down_eff @ h                                            # [2048, 512]

Sharding (8 cores, tensor-parallel on d_ff): core c owns d_ff rows
[c*1024, (c+1)*1024) of gate/up (and the matching columns of down).
Each core computes a full-[2048, 512] partial of the down projection;
partials are summed on the host (the all-reduce step).

Device notes:
  - PE matmul computes psum[M,N] = lhsT[K,M].T @ rhs[K,N] with K on
    partitions, so every weight is fed with its contraction dim on
    partitions. The host pre-transposes the snapped weights (one numpy
    transpose each) because fp32 has no DMA-transpose path on TRN2.
  - The fp32 snapped weights (24 MiB/core — the dominant HBM traffic)
    stream through in [128, 2, 512] pairs: one 512 KiB DMA, a packed
    pair of rank-32 scale matmuls (row-tiled via tile_position so both
    run concurrently in the PE array), one DVE dequant multiply that
    rounds to bf16, then eight [128,128]x[128,512] bf16 main matmuls
    with fp32 psum accumulation. bf16 streams ~3x faster than fp32r on
    the PE, which is what makes the kernel DMA-bound. Measured
    end-to-end error vs the fp32 reference: ~5e-3 of output absmax.
  - The broadcast activations x and the rank-32 factors are shipped
    bf16 in their final device layouts (host prep), so no on-device
    staging/rounding chain exists to stall the weight pipeline.
  - DMA rings: sync HWDGE carries only the weight stream (HWDGE is
    FIFO per issuing engine — a waiting DMA would head-of-line block
    the stream), scalar HWDGE carries the small constant loads, and
    output stores go out the gpsimd SWDGE ring.
  - Each pass's first scale-pack/dequant is emitted before the
    previous pass's epilogue so pass boundaries only wait on psum
    accumulator release.
"""

import numpy as np
import ml_dtypes

import concourse.bass as bass
from concourse import bacc
import concourse.mybir as mybir
from concourse.tile import TileContext
from concourse.bass_utils import run_bass_kernel_spmd

P = 128
D = 2048        # d_model
FF = 8192       # d_ff (global)
S = 512         # sequence
R = 32          # rank
NCORES = 8
F = FF // NCORES          # 1024 local d_ff rows
KD = D // P               # 16 d_model chunks
KF = F // P               # 8 local d_ff chunks
FG = 512                  # free-dim group (psum bank width)

f32 = mybir.dt.float32
bf16 = mybir.dt.bfloat16

_CACHE = {}


def _build():
    nc = bacc.Bacc()
    # x / scale factors arrive bf16 in device layout; weights arrive fp32.
    x = nc.declare_dram_parameter("x", [D, S], bf16, isOutput=False)
    gT = nc.declare_dram_parameter("gT", [D, F], f32, isOutput=False)
    uT = nc.declare_dram_parameter("uT", [D, F], f32, isOutput=False)
    dT = nc.declare_dram_parameter("dT", [F, D], f32, isOutput=False)
    # B2 [64, nk/2, 128]: strip i holds B cols for kd-chunk 2*kp+i (lhsT of
    # the packed scale matmul); AT2 [64, w]: A^T replicated on both strips.
    gB2 = nc.declare_dram_parameter("gB2", [2 * R, KD // 2, P], bf16, isOutput=False)
    uB2 = nc.declare_dram_parameter("uB2", [2 * R, KD // 2, P], bf16, isOutput=False)
    dB2 = nc.declare_dram_parameter("dB2", [2 * R, KF // 2, P], bf16, isOutput=False)
    gAT2 = nc.declare_dram_parameter("gAT2", [2 * R, F], bf16, isOutput=False)
    uAT2 = nc.declare_dram_parameter("uAT2", [2 * R, F], bf16, isOutput=False)
    dAT2 = nc.declare_dram_parameter("dAT2", [2 * R, D], bf16, isOutput=False)
    out = nc.declare_dram_parameter("out", [D, S], f32, isOutput=True)

    with TileContext(nc) as tc:
        with (
            tc.tile_pool(name="const", bufs=1) as const,
            tc.tile_pool(name="wstream", bufs=14) as wpool,
            tc.tile_pool(name="hbuf", bufs=1) as hpool,
            tc.tile_pool(name="obuf", bufs=3) as opool,
            tc.tile_pool(name="psacc", bufs=1, space="PSUM") as psacc,
            tc.tile_pool(name="pssc", bufs=2, space="PSUM") as pssc,
        ):
            # Startup critical path: the first scale-pack needs the gate
            # factors and the first mains need x chunk 0, so those lead the
            # sync ring right before the weight stream; everything else
            # loads on the scalar ring.
            rounded = {}

            def load_factor(nm, dram, eng):
                rt = const.tile(list(dram.shape), bf16, name=f"{nm}r", tag=f"{nm}r")
                eng.dma_start(rt, dram[:])
                rounded[nm] = rt

            load_factor("gB", gB2, nc.sync)
            load_factor("gAT", gAT2, nc.scalar)

            XC = 2
            x_sb = [None] * (KD // XC)

            def load_x_chunk(q, eng):
                xt = const.tile([P, XC, S], bf16, name=f"x{q}", tag=f"x{q}")
                eng.dma_start(
                    xt, x[q * XC * P:(q + 1) * XC * P, :].rearrange(
                        "(ko p) s -> p ko s", p=P))
                x_sb[q] = xt

            def xs(kd):
                return x_sb[kd // XC][:, kd % XC]

            load_x_chunk(0, nc.scalar)
            load_x_chunk(1, nc.scalar)

            load_factor("uB", uB2, nc.gpsimd)
            load_factor("uAT", uAT2, nc.gpsimd)
            load_factor("dBs", dB2, nc.gpsimd)
            load_factor("dAT", dAT2, nc.gpsimd)

            # h = silu(gate) * up, [128, 8, 512] resident
            h_sb = hpool.tile([P, KF, S], bf16)

            silu = mybir.ActivationFunctionType.Silu

            def gate_up_finish(acc, fg, is_up):
                for fi in range(4):
                    f = fg * 4 + fi
                    if is_up:
                        nc.vector.tensor_mul(
                            out=h_sb[:, f], in0=h_sb[:, f], in1=acc[fi])
                    else:
                        nc.scalar.activation(h_sb[:, f], acc[fi], silu)

            def down_finish(acc, mg):
                # two batched [128, 2, 512] stores per pass; copies split
                # across ACT and DVE so the epilogue drains in ~1.4us. The
                # copy runs immediately before its store on the same program
                # position, so the HWDGE store can't head-of-line block the
                # remaining weight stream for long.
                for half in range(2):
                    ot2 = opool.tile([P, 2, S], f32, name="ot", tag="ot")
                    for j in range(2):
                        mi = half * 2 + j
                        if mi % 2 == 0:
                            nc.scalar.copy(ot2[:, j], acc[mi])
                        else:
                            nc.vector.tensor_copy(out=ot2[:, j], in_=acc[mi])
                    weng = nc.sync if half == 0 else nc.scalar
                    weng.dma_start(
                        out[(mg * 4 + half * 2) * P:
                            (mg * 4 + half * 2 + 2) * P, :].rearrange(
                            "(mo p) s -> p mo s", p=P), ot2)

            passes = []
            for is_up in (0, 1):
                for fg in range(F // FG):
                    passes.append(dict(
                        wdram=uT if is_up else gT,
                        Bn="uB" if is_up else "gB",
                        An="uAT" if is_up else "gAT",
                        nk=KD, fg=fg, rhs_fn=xs,
                        finish=lambda acc, fg=fg, is_up=is_up:
                            gate_up_finish(acc, fg, is_up),
                    ))
            for mg in range(D // FG):
                passes.append(dict(
                    wdram=dT, Bn="dBs", An="dAT",
                    nk=KF, fg=mg, rhs_fn=lambda kf: h_sb[:, kf],
                    finish=lambda acc, mg=mg: down_finish(acc, mg),
                ))

            sc_tiles = {}

            def emit_sc(pi, kp):
                ps = passes[pi]
                fg = ps["fg"]
                sc2 = pssc.tile([P, 2, FG], f32, name="sc", tag="sc")
                for i in range(2):
                    nc.tensor.matmul(
                        sc2[:, i],
                        rounded[ps["Bn"]][i * R:(i + 1) * R, kp],
                        rounded[ps["An"]][i * R:(i + 1) * R,
                                          fg * FG:(fg + 1) * FG],
                        start=True, stop=True,
                        tile_position=(R * i, 0),
                    )
                sc_tiles[pi, kp] = sc2

            wr_tiles = {}

            def emit_wt_dequant(pi, kp):
                """Weight DMA + dequant multiply for pair (pi, kp); the wr
                tile is what the main matmuls consume."""
                ps = passes[pi]
                fg = ps["fg"]
                wt2 = wpool.tile([P, 2, FG], f32, name="wt", tag="wt")
                # alternate the weight stream across both HWDGE rings so
                # back-to-back pairs overlap their transfer+completion
                # latency instead of serializing on one ring.
                weng = nc.sync if (pi + kp) % 2 == 0 else nc.scalar
                weng.dma_start(
                    wt2,
                    ps["wdram"][kp * 2 * P:(kp + 1) * 2 * P,
                                fg * FG:(fg + 1) * FG].rearrange(
                                    "(ko p) f -> p ko f", p=P))
                wr2 = wpool.tile([P, 2, FG], bf16, name="wr", tag="wr")
                nc.vector.tensor_mul(out=wr2, in0=wt2,
                                     in1=sc_tiles.pop((pi, kp)))
                wr_tiles[pi, kp] = wr2

            emit_sc(0, 0)
            emit_wt_dequant(0, 0)
            for pi, ps in enumerate(passes):
                npairs = ps["nk"] // 2
                rhs_fn = ps["rhs_fn"]
                acc = [psacc.tile([P, S], f32, name=f"acc{i}", tag=f"acc{i}")
                       for i in range(4)]
                for kp in range(npairs):
                    if pi == 0 and 2 <= kp + 2 < KD // XC:
                        # pull the rest of x in just-in-time on the ring the
                        # weight stream isn't using this iteration (chunk q
                        # is first consumed at pair kp=q).
                        load_x_chunk(
                            kp + 2,
                            nc.scalar if (pi + kp) % 2 == 0 else nc.sync)
                    # look-ahead: scale pack for the pair after next's
                    # dequant, weight+dequant for the next pair — crossing
                    # into the next pass at the end so its pipeline is
                    # already primed when this pass's epilogue runs.
                    if kp + 1 < npairs:
                        emit_sc(pi, kp + 1)
                        emit_wt_dequant(pi, kp + 1)
                    elif pi + 1 < len(passes):
                        emit_sc(pi + 1, 0)
                        emit_wt_dequant(pi + 1, 0)
                    wr2 = wr_tiles.pop((pi, kp))
                    for j in range(2):
                        for fi in range(4):
                            nc.tensor.matmul(
                                acc[fi],
                                wr2[:, j, fi * P:(fi + 1) * P],
                                rhs_fn(2 * kp + j),
                                start=(kp == 0 and j == 0),
                                stop=(kp == npairs - 1 and j == 1),
                            )
                ps["finish"](acc)
    nc.finalize()
    return nc


def _prep_inputs(x, gate_snapped, gate_scale_A, gate_scale_B,
                 up_snapped, up_scale_A, up_scale_B,
                 down_snapped, down_scale_A, down_scale_B):
    asf = lambda a: np.ascontiguousarray(np.asarray(a, dtype=np.float32))
    bf = ml_dtypes.bfloat16
    x2 = np.ascontiguousarray(np.asarray(x, dtype=np.float32).reshape(D, S)
                              .astype(bf))
    gT_full = asf(gate_snapped).T      # [D, FF] view
    uT_full = asf(up_snapped).T
    dT_full = asf(down_snapped).T      # [FF, D] view

    def pack_B2(Bmat, nk):
        # [R, nk*128] -> [64, nk/2, 128]: strip i holds chunks 2*kp+i
        b = np.asarray(Bmat, dtype=np.float32).reshape(R, nk // 2, 2, P)
        o = np.empty((2 * R, nk // 2, P), dtype=bf)
        o[:R] = b[:, :, 0, :].astype(bf)
        o[R:] = b[:, :, 1, :].astype(bf)
        return o

    def pack_AT2(Amat):
        # A [w, R] -> A^T [R, w] replicated on both strips -> [64, w]
        at = np.asarray(Amat, dtype=np.float32).T.astype(bf)
        return np.ascontiguousarray(np.concatenate([at, at], axis=0))

    gB_f = np.asarray(gate_scale_B, dtype=np.float32)
    uB_f = np.asarray(up_scale_B, dtype=np.float32)
    dB_f = np.asarray(down_scale_B, dtype=np.float32)
    gA_f = np.asarray(gate_scale_A, dtype=np.float32)
    uA_f = np.asarray(up_scale_A, dtype=np.float32)
    dAT2 = pack_AT2(down_scale_A)      # [64, D]

    in_maps = []
    for c in range(NCORES):
        lo, hi = c * F, (c + 1) * F
        in_maps.append({
            "x": x2,
            "gT": np.ascontiguousarray(gT_full[:, lo:hi]),
            "uT": np.ascontiguousarray(uT_full[:, lo:hi]),
            "dT": np.ascontiguousarray(dT_full[lo:hi, :]),
            "gB2": pack_B2(gB_f, KD),
            "uB2": pack_B2(uB_f, KD),
            "dB2": pack_B2(dB_f[:, lo:hi], KF),
            "gAT2": pack_AT2(gA_f[lo:hi]),
            "uAT2": pack_AT2(uA_f[lo:hi]),
            "dAT2": dAT2,
        })
    return in_maps


def run(trace=False, **inputs):
    if "nc" not in _CACHE:
        _CACHE["nc"] = _build()
    nc = _CACHE["nc"]
    in_maps = _prep_inputs(**inputs)
    res = run_bass_kernel_spmd(nc, in_maps, list(range(NCORES)), trace=trace)
    partial = np.zeros((D, S), dtype=np.float32)
    for c in range(NCORES):
        partial += res.results[c]["out"]
    return partial.reshape(1, D, 1, S), res


def kernel(**inputs):
    out, _ = run(trace=False, **inputs)
    return out


if __name__ == "__main__":
    rng = np.random.default_rng(0)
    ins = {
        "x": rng.standard_normal((1, D, 1, S)).astype(np.float32),
        "gate_snapped": (rng.standard_normal((FF, D)) * 0.02).astype(np.float32),
        "gate_scale_A": (rng.standard_normal((FF, R)) * 0.1).astype(np.float32),
        "gate_scale_B": (rng.standard_normal((R, D)) * 0.1).astype(np.float32),
        "up_snapped": (rng.standard_normal((FF, D)) * 0.02).astype(np.float32),
        "up_scale_A": (rng.standard_normal((FF, R)) * 0.1).astype(np.float32),
        "up_scale_B": (rng.standard_normal((R, D)) * 0.1).astype(np.float32),
        "down_snapped": (rng.standard_normal((D, FF)) * 0.02).astype(np.float32),
        "down_scale_A": (rng.standard_normal((D, R)) * 0.1).astype(np.float32),
        "down_scale_B": (rng.standard_normal((R, FF)) * 0.1).astype(np.float32),
    }
    out = kernel(**ins)
    print("kernel ran, out shape", out.shape, "mean abs", np.abs(out).mean())


# revision 21
# speedup vs baseline: 1.0076x; 1.0076x over previous
"""Trainium2 Bass kernel for FFNWithScales (SwiGLU MLP with low-rank dequant scales).

Reference computation (all fp32):
    gate_eff = gate_snapped * (gate_scale_A @ gate_scale_B)       # [8192, 2048]
    up_eff   = up_snapped   * (up_scale_A   @ up_scale_B)         # [8192, 2048]
    down_eff = down_snapped * (down_scale_A @ down_scale_B)       # [2048, 8192]
    h   = silu(gate_eff @ x) * (up_eff @ x)                       # [8192, 512]
    out = down_eff @ h                                            # [2048, 512]

Sharding (8 cores, tensor-parallel on d_ff): core c owns d_ff rows
[c*1024, (c+1)*1024) of gate/up (and the matching columns of down).
Each core computes a full-[2048, 512] partial of the down projection;
partials are summed on the host (the all-reduce step).

Device notes:
  - PE matmul computes psum[M,N] = lhsT[K,M].T @ rhs[K,N] with K on
    partitions, so every weight is fed with its contraction dim on
    partitions. The host pre-transposes the snapped weights (one numpy
    transpose each) because fp32 has no DMA-transpose path on TRN2.
  - The fp32 snapped weights (24 MiB/core — the dominant HBM traffic)
    stream through in [128, 2, 512] pairs: one 512 KiB DMA, a packed
    pair of rank-32 scale matmuls (row-tiled via tile_position so both
    run concurrently in the PE array), one DVE dequant multiply that
    rounds to bf16, then eight [128,128]x[128,512] bf16 main matmuls
    with fp32 psum accumulation. bf16 streams ~3x faster than fp32r on
    the PE, which is what makes the kernel DMA-bound. Measured
    end-to-end error vs the fp32 reference: ~5e-3 of output absmax.
  - The broadcast activations x and the rank-32 factors are shipped
    bf16 in their final device layouts (host prep), so no on-device
    staging/rounding chain exists to stall the weight pipeline.
  - DMA rings: sync HWDGE carries only the weight stream (HWDGE is
    FIFO per issuing engine — a waiting DMA would head-of-line block
    the stream), scalar HWDGE carries the small constant loads, and
    output stores go out the gpsimd SWDGE ring.
  - Each pass's first scale-pack/dequant is emitted before the
    previous pass's epilogue so pass boundaries only wait on psum
    accumulator release.
"""

import numpy as np
import ml_dtypes

import concourse.bass as bass
from concourse import bacc
import concourse.mybir as mybir
from concourse.tile import TileContext
from concourse.bass_utils import run_bass_kernel_spmd

P = 128
D = 2048        # d_model
FF = 8192       # d_ff (global)
S = 512         # sequence
R = 32          # rank
NCORES = 8
F = FF // NCORES          # 1024 local d_ff rows
KD = D // P               # 16 d_model chunks
KF = F // P               # 8 local d_ff chunks
FG = 512                  # free-dim group (psum bank width)

f32 = mybir.dt.float32
bf16 = mybir.dt.bfloat16

_CACHE = {}


def _build():
    nc = bacc.Bacc()
    # x / scale factors arrive bf16 in device layout; weights arrive fp32.
    x = nc.declare_dram_parameter("x", [D, S], bf16, isOutput=False)
    gT = nc.declare_dram_parameter("gT", [D, F], f32, isOutput=False)
    uT = nc.declare_dram_parameter("uT", [D, F], f32, isOutput=False)
    dT = nc.declare_dram_parameter("dT", [F, D], f32, isOutput=False)
    # B2 [64, nk/2, 128]: strip i holds B cols for kd-chunk 2*kp+i (lhsT of
    # the packed scale matmul); AT2 [64, w]: A^T replicated on both strips.
    gB2 = nc.declare_dram_parameter("gB2", [2 * R, KD // 2, P], bf16, isOutput=False)
    uB2 = nc.declare_dram_parameter("uB2", [2 * R, KD // 2, P], bf16, isOutput=False)
    dB2 = nc.declare_dram_parameter("dB2", [2 * R, KF // 2, P], bf16, isOutput=False)
    gAT2 = nc.declare_dram_parameter("gAT2", [2 * R, F], bf16, isOutput=False)
    uAT2 = nc.declare_dram_parameter("uAT2", [2 * R, F], bf16, isOutput=False)
    dAT2 = nc.declare_dram_parameter("dAT2", [2 * R, D], bf16, isOutput=False)
    out = nc.declare_dram_parameter("out", [D, S], f32, isOutput=True)

    with TileContext(nc) as tc:
        with (
            tc.tile_pool(name="const", bufs=1) as const,
            tc.tile_pool(name="wstream", bufs=14) as wpool,
            tc.tile_pool(name="hbuf", bufs=1) as hpool,
            tc.tile_pool(name="obuf", bufs=3) as opool,
            tc.tile_pool(name="psacc", bufs=1, space="PSUM") as psacc,
            tc.tile_pool(name="pssc", bufs=2, space="PSUM") as pssc,
        ):
            # Startup critical path: the first scale-pack needs the gate
            # factors and the first mains need x chunk 0, so those lead the
            # sync ring right before the weight stream; everything else
            # loads on the scalar ring.
            rounded = {}

            def load_factor(nm, dram, eng):
                rt = const.tile(list(dram.shape), bf16, name=f"{nm}r", tag=f"{nm}r")
                eng.dma_start(rt, dram[:])
                rounded[nm] = rt

            load_factor("gB", gB2, nc.sync)
            load_factor("gAT", gAT2, nc.scalar)

            XC = 2
            x_sb = [None] * (KD // XC)

            def load_x_chunk(q, eng):
                xt = const.tile([P, XC, S], bf16, name=f"x{q}", tag=f"x{q}")
                eng.dma_start(
                    xt, x[q * XC * P:(q + 1) * XC * P, :].rearrange(
                        "(ko p) s -> p ko s", p=P))
                x_sb[q] = xt

            def xs(kd):
                return x_sb[kd // XC][:, kd % XC]

            load_x_chunk(0, nc.scalar)
            load_x_chunk(1, nc.scalar)

            load_factor("uB", uB2, nc.gpsimd)
            load_factor("uAT", uAT2, nc.gpsimd)
            load_factor("dBs", dB2, nc.gpsimd)
            load_factor("dAT", dAT2, nc.gpsimd)

            # h = silu(gate) * up, [128, 8, 512] resident
            h_sb = hpool.tile([P, KF, S], bf16)

            silu = mybir.ActivationFunctionType.Silu

            def gate_up_finish(acc, fg, is_up):
                for fi in range(4):
                    f = fg * 4 + fi
                    if is_up:
                        nc.vector.tensor_mul(
                            out=h_sb[:, f], in0=h_sb[:, f], in1=acc[fi])
                    else:
                        nc.scalar.activation(h_sb[:, f], acc[fi], silu)

            def down_finish(acc, mg):
                # two batched [128, 2, 512] stores per pass; copies split
                # across ACT and DVE so the epilogue drains in ~1.4us. The
                # copy runs immediately before its store on the same program
                # position, so the HWDGE store can't head-of-line block the
                # remaining weight stream for long.
                for half in range(2):
                    ot2 = opool.tile([P, 2, S], f32, name="ot", tag="ot")
                    for j in range(2):
                        nc.scalar.copy(ot2[:, j], acc[half * 2 + j])
                    weng = nc.sync if half == 0 else nc.scalar
                    weng.dma_start(
                        out[(mg * 4 + half * 2) * P:
                            (mg * 4 + half * 2 + 2) * P, :].rearrange(
                            "(mo p) s -> p mo s", p=P), ot2)

            passes = []
            for is_up in (0, 1):
                for fg in range(F // FG):
                    passes.append(dict(
                        wdram=uT if is_up else gT,
                        Bn="uB" if is_up else "gB",
                        An="uAT" if is_up else "gAT",
                        nk=KD, fg=fg, rhs_fn=xs,
                        finish=lambda acc, fg=fg, is_up=is_up:
                            gate_up_finish(acc, fg, is_up),
                    ))
            for mg in range(D // FG):
                passes.append(dict(
                    wdram=dT, Bn="dBs", An="dAT",
                    nk=KF, fg=mg, rhs_fn=lambda kf: h_sb[:, kf],
                    finish=lambda acc, mg=mg: down_finish(acc, mg),
                ))

            sc_tiles = {}

            def emit_sc(pi, kp):
                ps = passes[pi]
                fg = ps["fg"]
                sc2 = pssc.tile([P, 2, FG], f32, name="sc", tag="sc")
                for i in range(2):
                    nc.tensor.matmul(
                        sc2[:, i],
                        rounded[ps["Bn"]][i * R:(i + 1) * R, kp],
                        rounded[ps["An"]][i * R:(i + 1) * R,
                                          fg * FG:(fg + 1) * FG],
                        start=True, stop=True,
                        tile_position=(R * i, 0),
                    )
                sc_tiles[pi, kp] = sc2

            wr_tiles = {}

            def emit_wt_dequant(pi, kp):
                """Weight DMA + dequant multiply for pair (pi, kp); the wr
                tile is what the main matmuls consume."""
                ps = passes[pi]
                fg = ps["fg"]
                wt2 = wpool.tile([P, 2, FG], f32, name="wt", tag="wt")
                # alternate the weight stream across both HWDGE rings so
                # back-to-back pairs overlap their transfer+completion
                # latency instead of serializing on one ring. The first two
                # pairs are on the latency-critical startup path: split each
                # across both rings so the halves transfer concurrently.
                src = ps["wdram"][kp * 2 * P:(kp + 1) * 2 * P,
                                  fg * FG:(fg + 1) * FG].rearrange(
                                      "(ko p) f -> p ko f", p=P)
                if pi == 0 and kp < 2:
                    nc.sync.dma_start(wt2[:, 0], src[:, 0])
                    nc.scalar.dma_start(wt2[:, 1], src[:, 1])
                else:
                    weng = nc.sync if (pi + kp) % 2 == 0 else nc.scalar
                    weng.dma_start(wt2, src)
                wr2 = wpool.tile([P, 2, FG], bf16, name="wr", tag="wr")
                nc.vector.tensor_mul(out=wr2, in0=wt2,
                                     in1=sc_tiles.pop((pi, kp)))
                wr_tiles[pi, kp] = wr2

            emit_sc(0, 0)
            emit_wt_dequant(0, 0)
            for pi, ps in enumerate(passes):
                npairs = ps["nk"] // 2
                rhs_fn = ps["rhs_fn"]
                acc = [psacc.tile([P, S], f32, name=f"acc{i}", tag=f"acc{i}")
                       for i in range(4)]
                for kp in range(npairs):
                    if pi == 0 and 2 <= kp + 2 < KD // XC:
                        # pull the rest of x in just-in-time on the ring the
                        # weight stream isn't using this iteration (chunk q
                        # is first consumed at pair kp=q).
                        load_x_chunk(
                            kp + 2,
                            nc.scalar if (pi + kp) % 2 == 0 else nc.sync)
                    # look-ahead: scale pack for the pair after next's
                    # dequant, weight+dequant for the next pair — crossing
                    # into the next pass at the end so its pipeline is
                    # already primed when this pass's epilogue runs.
                    if kp + 1 < npairs:
                        emit_sc(pi, kp + 1)
                        emit_wt_dequant(pi, kp + 1)
                    elif pi + 1 < len(passes):
                        emit_sc(pi + 1, 0)
                        emit_wt_dequant(pi + 1, 0)
                    wr2 = wr_tiles.pop((pi, kp))
                    for j in range(2):
                        for fi in range(4):
                            nc.tensor.matmul(
                                acc[fi],
                                wr2[:, j, fi * P:(fi + 1) * P],
                                rhs_fn(2 * kp + j),
                                start=(kp == 0 and j == 0),
                                stop=(kp == npairs - 1 and j == 1),
                            )
                ps["finish"](acc)
    nc.finalize()
    return nc


def _prep_inputs(x, gate_snapped, gate_scale_A, gate_scale_B,
                 up_snapped, up_scale_A, up_scale_B,
                 down_snapped, down_scale_A, down_scale_B):
    asf = lambda a: np.ascontiguousarray(np.asarray(a, dtype=np.float32))
    bf = ml_dtypes.bfloat16
    x2 = np.ascontiguousarray(np.asarray(x, dtype=np.float32).reshape(D, S)
                              .astype(bf))
    gT_full = asf(gate_snapped).T      # [D, FF] view
    uT_full = asf(up_snapped).T
    dT_full = asf(down_snapped).T      # [FF, D] view

    def pack_B2(Bmat, nk):
        # [R, nk*128] -> [64, nk/2, 128]: strip i holds chunks 2*kp+i
        b = np.asarray(Bmat, dtype=np.float32).reshape(R, nk // 2, 2, P)
        o = np.empty((2 * R, nk // 2, P), dtype=bf)
        o[:R] = b[:, :, 0, :].astype(bf)
        o[R:] = b[:, :, 1, :].astype(bf)
        return o

    def pack_AT2(Amat):
        # A [w, R] -> A^T [R, w] replicated on both strips -> [64, w]
        at = np.asarray(Amat, dtype=np.float32).T.astype(bf)
        return np.ascontiguousarray(np.concatenate([at, at], axis=0))

    gB_f = np.asarray(gate_scale_B, dtype=np.float32)
    uB_f = np.asarray(up_scale_B, dtype=np.float32)
    dB_f = np.asarray(down_scale_B, dtype=np.float32)
    gA_f = np.asarray(gate_scale_A, dtype=np.float32)
    uA_f = np.asarray(up_scale_A, dtype=np.float32)
    dAT2 = pack_AT2(down_scale_A)      # [64, D]

    in_maps = []
    for c in range(NCORES):
        lo, hi = c * F, (c + 1) * F
        in_maps.append({
            "x": x2,
            "gT": np.ascontiguousarray(gT_full[:, lo:hi]),
            "uT": np.ascontiguousarray(uT_full[:, lo:hi]),
            "dT": np.ascontiguousarray(dT_full[lo:hi, :]),
            "gB2": pack_B2(gB_f, KD),
            "uB2": pack_B2(uB_f, KD),
            "dB2": pack_B2(dB_f[:, lo:hi], KF),
            "gAT2": pack_AT2(gA_f[lo:hi]),
            "uAT2": pack_AT2(uA_f[lo:hi]),
            "dAT2": dAT2,
        })
    return in_maps


def run(trace=False, **inputs):
    if "nc" not in _CACHE:
        _CACHE["nc"] = _build()
    nc = _CACHE["nc"]
    in_maps = _prep_inputs(**inputs)
    res = run_bass_kernel_spmd(nc, in_maps, list(range(NCORES)), trace=trace)
    partial = np.zeros((D, S), dtype=np.float32)
    for c in range(NCORES):
        partial += res.results[c]["out"]
    return partial.reshape(1, D, 1, S), res


def kernel(**inputs):
    out, _ = run(trace=False, **inputs)
    return out


if __name__ == "__main__":
    rng = np.random.default_rng(0)
    ins = {
        "x": rng.standard_normal((1, D, 1, S)).astype(np.float32),
        "gate_snapped": (rng.standard_normal((FF, D)) * 0.02).astype(np.float32),
        "gate_scale_A": (rng.standard_normal((FF, R)) * 0.1).astype(np.float32),
        "gate_scale_B": (rng.standard_normal((R, D)) * 0.1).astype(np.float32),
        "up_snapped": (rng.standard_normal((FF, D)) * 0.02).astype(np.float32),
        "up_scale_A": (rng.standard_normal((FF, R)) * 0.1).astype(np.float32),
        "up_scale_B": (rng.standard_normal((R, D)) * 0.1).astype(np.float32),
        "down_snapped": (rng.standard_normal((D, FF)) * 0.02).astype(np.float32),
        "down_scale_A": (rng.standard_normal((D, R)) * 0.1).astype(np.float32),
        "down_scale_B": (rng.standard_normal((R, FF)) * 0.1).astype(np.float32),
    }
    out = kernel(**ins)
    print("kernel ran, out shape", out.shape, "mean abs", np.abs(out).mean())
